# revision 7
# baseline (speedup 1.0000x reference)
"""TRN2 Bass kernel for FFQLinear: y = x @ ((q - zp) * scale) + bias.

x: [2, 2048, 4096] f32, q: [4096, 4096] int32 (values 0..255),
scale/zero_point: [1] f32, bias: [4096] f32 -> y: [2, 2048, 4096] f32.

Strategy (8 NeuronCores, M split 8 ways, weight replicated), fp8
DoubleRow matmuls:
  - Exact split q = q_hi16 + q_lo (both e4m3-exact); x ~ x_hi + x_lo
    (two e4m3 terms, ~0.4% residual);
    acc = (x_hi + x_lo) @ q_hi16 + x_hi @ q_lo  (3 fp8 GEMM passes).
  - DoubleRow disables FWL: its LDWEIGHTS loads 256 columns (~213ns)
    vs ~120ns matmul exec at N=512, so self-loading matmuls are
    LDWEIGHTS-bound (~215ns/MM measured).  v3 therefore PAIRS matmuls
    on the same stationary tile: panels are 1024 wide and each
    (x-tile, k-pair) stationary feeds two 512-col moving streams into
    two PSUM banks back-to-back, halving LDWEIGHTS traffic (relies on
    walrus eliding the repeated weight load).
  - Affine handled outside the GEMM: y = scale*acc - scale*zp*
    rowsum(x)[m] + bias[n]; corr precomputed on host.  Epilogue = one
    DVE tensor_scalar (acc*scale + corr) + one tensor_tensor (+bias,
    f16 out).
  - Host packs w into the exact SBUF panel layout [NPAN, P, KO, 2,
    NPAIR] (hi/lo DMA'd separately so pass 1 can start before lo
    lands); x packed [P, MT, KO, P].  Loads on the SP HWDGE queue,
    stores on the Act HWDGE queue (next-panel weight prefetch never
    queues behind output drain).
"""
import numpy as np


def _ensure_paths():
    import sys
    try:
        import concourse  # noqa: F401
        return
    except ImportError:
        pass
    for p in ("/opt/trn_rl_repo", "/root/.axon_site/_ro/trn_rl_repo"):
        if p not in sys.path:
            sys.path.insert(0, p)
    import concourse  # noqa: F401


B, S, DIN, DOUT = 2, 2048, 4096, 4096
N_CORES = 8
M_SH = (B * S) // N_CORES        # 512 rows per core
P = 128
KO = DIN // P                    # 32 k-tiles of 128
KP = KO // 2                     # 16 k-pairs per GEMM pass (DoubleRow)
MT = M_SH // P                   # 4 m-tiles
NTILE = 512
NPAIR = 1024                     # w panel width (2 n-chunks)
NPAN = DOUT // NPAIR             # 4 weight panels


def _dedupe_ldweights(nc, mybir):
    """Drop InstLdweights whose weights AP matches the weights already
    loaded by the previous PE load (the tile scheduler splits every
    matmul into Ldweights + Matmult(ldweights=False) and never dedupes).
    DoubleRow LDWEIGHTS loads 256 columns (~213ns) vs ~107ns matmul exec
    at N=512, so pairing two matmuls per load moves the kernel from
    LDWEIGHTS-bound to exec-bound."""
    def key(i):
        ap = i.ins[0]
        return (ap.memref, ap.offset, tuple(map(tuple, ap.ap)),
                str(ap.dtype), str(i.perf_mode), str(i.is_transpose))

    removed = 0
    for blk in nc.main_func.blocks:
        last = None
        keep = []
        for inst in blk.instructions:
            if isinstance(inst, mybir.InstLdweights):
                k = key(inst)
                si = inst.sync_info
                clean = si is None or (len(si.on_wait) == 0
                                       and len(si.on_update) == 0)
                if k == last and clean:
                    removed += 1
                    continue
                last = k
            elif isinstance(inst, mybir.InstMatmult):
                pass  # matmuls consume but don't replace loaded weights
            elif getattr(inst, "engine", None) == mybir.EngineType.PE:
                last = None  # unknown PE instruction: be conservative
            keep.append(inst)
        blk.instructions[:] = keep
    return removed


def _build(reps: int = 1):
    from contextlib import ExitStack
    import concourse.bass as bass
    import concourse.tile as tile
    from concourse import bacc, mybir
    from concourse.bass import ts

    f32 = mybir.dt.float32
    f16 = mybir.dt.float16
    f8 = mybir.dt.float8e4
    DR = mybir.MatmulPerfMode.DoubleRow

    nc = bacc.Bacc("TRN2", target_bir_lowering=False, debug=False)

    xh = nc.dram_tensor("xh", [P, MT, KO, P], f8, kind="ExternalInput")
    xl = nc.dram_tensor("xl", [P, MT, KO, P], f8, kind="ExternalInput")
    whl = nc.dram_tensor("whl", [NPAN, P, KO, 2, NPAIR], f8,
                         kind="ExternalInput")
    biass = nc.dram_tensor("biass", [DOUT], f32, kind="ExternalInput")
    scv = nc.dram_tensor("scv", [P, 1 + MT], f32, kind="ExternalInput")
    ys = nc.dram_tensor("ys", [M_SH, DOUT], f16, kind="ExternalOutput")

    with tile.TileContext(nc) as tc, ExitStack() as ctx:
        x_pool = ctx.enter_context(tc.tile_pool(name="x_pool", bufs=1))
        w_pool = ctx.enter_context(tc.tile_pool(name="w_pool", bufs=2))
        b_pool = ctx.enter_context(tc.tile_pool(name="b_pool", bufs=2))
        s_pool = ctx.enter_context(tc.tile_pool(name="s_pool", bufs=1))
        t_pool = ctx.enter_context(tc.tile_pool(name="t_pool", bufs=2))
        y_pool = ctx.enter_context(tc.tile_pool(name="y_pool", bufs=2))
        psum = ctx.enter_context(
            tc.tile_pool(name="psum", bufs=4, space="PSUM"))

        def body():
            sct = s_pool.tile([P, 1 + MT], f32, tag="sc")
            xht = x_pool.tile([P, MT, KO, P], f8, tag="xh")
            xlt = x_pool.tile([P, MT, KO, P], f8, tag="xl")
            nc.sync.dma_start(sct[:], scv[:])
            nc.sync.dma_start(xht[:, 0], xh[:, 0])
            nc.sync.dma_start(xlt[:, 0], xl[:, 0])

            for pa in range(NPAN):
                wt = w_pool.tile([P, KO, 2, NPAIR], f8, tag="w")
                # hi first: passes 1+2 only need hi, lo lands during them
                nc.sync.dma_start(wt[:, :, 0], whl[pa, :, :, 0])
                nc.sync.dma_start(wt[:, :, 1], whl[pa, :, :, 1])
                bt = b_pool.tile([P, NPAIR], f32, tag="bias")
                nc.sync.dma_start(
                    bt[:], biass[ts(pa, NPAIR)].partition_broadcast(P))
                if pa == 0:
                    for mi in range(1, MT):
                        nc.sync.dma_start(xht[:, mi], xh[:, mi])
                        nc.sync.dma_start(xlt[:, mi], xl[:, mi])
                for mi in range(MT):
                    accA = psum.tile([P, NTILE], f32, tag="accA",
                                     name=f"accA_{pa}_{mi}")
                    accB = psum.tile([P, NTILE], f32, tag="accB",
                                     name=f"accB_{pa}_{mi}")
                    for ps, (xt_, h) in enumerate(
                            ((xht, 0), (xlt, 0), (xht, 1))):
                        for ki in range(KP):
                            first = (ps == 0 and ki == 0)
                            last = (ps == 2 and ki == KP - 1)
                            lhs = xt_[:, mi, 2 * ki:2 * ki + 2]
                            nc.tensor.matmul(
                                accA[:], lhsT=lhs,
                                rhs=wt[:, 2 * ki:2 * ki + 2, h, 0:NTILE],
                                start=first, stop=last, perf_mode=DR)
                            nc.tensor.matmul(
                                accB[:], lhsT=lhs,
                                rhs=wt[:, 2 * ki:2 * ki + 2, h,
                                       NTILE:NPAIR],
                                start=first, stop=last, perf_mode=DR)
                    for sub, acc in ((0, accA), (1, accB)):
                        tt = t_pool.tile([P, NTILE], f32, tag="t")
                        nc.vector.tensor_scalar(
                            tt[:], acc[:], sct[:, 0:1],
                            sct[:, 1 + mi:2 + mi],
                            mybir.AluOpType.mult, mybir.AluOpType.add)
                        yt = y_pool.tile([P, NTILE], f16, tag="y")
                        nc.vector.tensor_tensor(
                            yt[:], tt[:], bt[:, ts(sub, NTILE)],
                            mybir.AluOpType.add)
                        nc.scalar.dma_start(
                            ys[ts(mi, P),
                               pa * NPAIR + sub * NTILE:
                               pa * NPAIR + (sub + 1) * NTILE], yt[:])

        if reps == 1:
            body()
        else:
            with tc.For_i(0, reps, 1):
                body()

    _dedupe_ldweights(nc, mybir)
    nc.compile()
    return nc


def prep_inputs(x: np.ndarray, q_int_weight: np.ndarray, scale: np.ndarray,
                zero_point: np.ndarray, bias: np.ndarray):
    """Host-side prep: exact fp8 split of q, 2-term fp8 split of x,
    rowsum correction, panel-layout packing."""
    import ml_dtypes
    f8 = ml_dtypes.float8_e4m3

    scale_f = np.float32(np.asarray(scale).reshape(-1)[0])
    zp_f = np.float32(np.asarray(zero_point).reshape(-1)[0])

    q = np.asarray(q_int_weight)
    # [pa, p, ko, {hi,lo}, n]; element = q part at (k=ko*128+p, col=pa*NPAIR+n)
    whl = np.empty((NPAN, P, KO, 2, NPAIR), dtype=f8)
    qh = (q & ~np.int32(15)).astype(np.float32).reshape(KO, P, NPAN, NPAIR)
    ql = (q & np.int32(15)).astype(np.float32).reshape(KO, P, NPAN, NPAIR)
    whl[:, :, :, 0, :] = qh.transpose(2, 1, 0, 3).astype(f8)
    whl[:, :, :, 1, :] = ql.transpose(2, 1, 0, 3).astype(f8)

    bf = np.ascontiguousarray(bias.astype(np.float32))
    xf = np.asarray(x, dtype=np.float32).reshape(B * S, DIN)

    def pack_x(v):  # [M_SH, DIN] -> [p, mi, ko, mc]
        return np.ascontiguousarray(
            v.T.reshape(KO, P, MT, P).transpose(1, 2, 0, 3))

    in_maps = []
    for c in range(N_CORES):
        xs = xf[c * M_SH:(c + 1) * M_SH]
        xh8 = xs.astype(f8)
        xl8 = (xs - xh8.astype(np.float32)).astype(f8)
        rsum = xs.astype(np.float64).sum(axis=1).astype(np.float32)
        scv = np.empty((P, 1 + MT), np.float32)
        scv[:, 0] = scale_f
        scv[:, 1:] = (-scale_f * zp_f) * rsum.reshape(MT, P).T
        in_maps.append({"xh": pack_x(xh8), "xl": pack_x(xl8), "whl": whl,
                        "biass": bf, "scv": scv})
    return in_maps


def kernel(x: np.ndarray, q_int_weight: np.ndarray, scale: np.ndarray,
           zero_point: np.ndarray, bias: np.ndarray) -> np.ndarray:
    _ensure_paths()
    from concourse.bass_utils import run_bass_kernel_spmd

    nc = _build()
    in_maps = prep_inputs(x, q_int_weight, scale, zero_point, bias)
    res = run_bass_kernel_spmd(nc, in_maps, core_ids=list(range(N_CORES)))

    y = np.empty((B * S, DOUT), np.float32)
    for c in range(N_CORES):
        y[c * M_SH:(c + 1) * M_SH] = res.results[c]["ys"].astype(np.float32)
    return y.reshape(B, S, DOUT)


# revision 8
# speedup vs baseline: 1.3996x; 1.3996x over previous
"""TRN2 Bass kernel for FFQLinear: y = x @ ((q - zp) * scale) + bias.

x: [2, 2048, 4096] f32, q: [4096, 4096] int32 (values 0..255),
scale/zero_point: [1] f32, bias: [4096] f32 -> y: [2, 2048, 4096] f32.

Strategy (8 NeuronCores, M split 8 ways, dequantized weight
replicated), fp16 matmuls:
  - fp8 DoubleRow was measured and rejected: the exact-split
    decomposition needs 3 full GEMM passes, and 3 passes at 2x rate =
    1.5x one fp16 GEMM (351us measured vs 263us fp16 baseline);
    1-pass fp8 fails the 2e-2 gate (e4m3 weight error alone ~2x over
    budget).  fp16 peak for this GEMM is ~218us/core.
  - Host prep: w16 = ((q - zp) * scale) fp16 packed per-panel into
    the exact SBUF layout [NPAN, P, KO, NTILE] (one 32KB-contiguous
    line per partition per panel -> max DMA efficiency); x shard
    packed mi-major [P, MT, KO, P] fp16 (k on partitions, no on-chip
    transpose); bias f32.
  - Per core: resident x (double-buffered across reps), w streamed in
    8 panels of 512 cols (bufs=2).  32 PSUM groups (panel x mi),
    strictly sequential, 32 self-loading matmuls each (FWL active:
    128-col fp16 weights load at 2x and hide under the 512-col
    stream).
  - Input DMAs ride the SP HWDGE queue, output stores the Act HWDGE
    queue: next-panel weight prefetch is never queued behind this
    panel's output drain (the v1 kernel lost ~45us/rep there).
  - Epilogue: one DVE tensor_tensor (acc + bias) with f16 output
    (halves out-traffic; ~5e-4 rel err), upcast on host.
"""
import numpy as np


def _ensure_paths():
    import sys
    try:
        import concourse  # noqa: F401
        return
    except ImportError:
        pass
    for p in ("/opt/trn_rl_repo", "/root/.axon_site/_ro/trn_rl_repo"):
        if p not in sys.path:
            sys.path.insert(0, p)
    import concourse  # noqa: F401


B, S, DIN, DOUT = 2, 2048, 4096, 4096
N_CORES = 8
M_SH = (B * S) // N_CORES        # 512 rows per core
P = 128
KO = DIN // P                    # 32 k-tiles of 128
MT = M_SH // P                   # 4 m-tiles
NTILE = 512
NPAN = DOUT // NTILE             # 8 weight panels


def _build(reps: int = 1):
    from contextlib import ExitStack
    import concourse.bass as bass
    import concourse.tile as tile
    from concourse import bacc, mybir
    from concourse.bass import ts

    f32 = mybir.dt.float32
    f16 = mybir.dt.float16

    nc = bacc.Bacc("TRN2", target_bir_lowering=False, debug=False)

    xts = nc.dram_tensor("xts", [P, MT, KO, P], f16, kind="ExternalInput")
    wpk = nc.dram_tensor("wpk", [NPAN, P, KO, NTILE], f16,
                         kind="ExternalInput")
    biass = nc.dram_tensor("biass", [DOUT], f32, kind="ExternalInput")
    ys = nc.dram_tensor("ys", [M_SH, DOUT], f16, kind="ExternalOutput")

    with tile.TileContext(nc) as tc, ExitStack() as ctx:
        x_pool = ctx.enter_context(tc.tile_pool(name="x_pool", bufs=2))
        w_pool = ctx.enter_context(tc.tile_pool(name="w_pool", bufs=2))
        b_pool = ctx.enter_context(tc.tile_pool(name="b_pool", bufs=2))
        y_pool = ctx.enter_context(tc.tile_pool(name="y_pool", bufs=2))
        psum = ctx.enter_context(
            tc.tile_pool(name="psum", bufs=8, space="PSUM"))

        def body():
            xT = x_pool.tile([P, MT, KO, P], f16, tag="xT")
            nc.sync.dma_start(xT[:, 0], xts[:, 0])

            for pa in range(NPAN):
                wt = w_pool.tile([P, KO, NTILE], f16, tag="w")
                nc.sync.dma_start(wt[:], wpk[pa])
                bt = b_pool.tile([P, NTILE], f32, tag="bias")
                nc.sync.dma_start(
                    bt[:], biass[ts(pa, NTILE)].partition_broadcast(P))
                if pa == 0:
                    for mi in range(1, MT):
                        nc.sync.dma_start(xT[:, mi], xts[:, mi])
                for mi in range(MT):
                    acc = psum.tile([P, NTILE], f32, tag="acc",
                                    name=f"acc_{pa}_{mi}")
                    for ki in range(KO):
                        nc.tensor.matmul(
                            acc[:], lhsT=xT[:, mi, ki], rhs=wt[:, ki],
                            start=(ki == 0), stop=(ki == KO - 1))
                    yt = y_pool.tile([P, NTILE], f16, tag="y")
                    nc.vector.tensor_tensor(
                        yt[:], acc[:], bt[:], mybir.AluOpType.add)
                    # stores on the Act HWDGE queue: never block the SP
                    # queue's next-panel weight prefetch
                    nc.scalar.dma_start(
                        ys[ts(mi, P), ts(pa, NTILE)], yt[:])

        if reps == 1:
            body()
        else:
            with tc.For_i(0, reps, 1):
                body()

    nc.compile()
    return nc


def prep_inputs(x: np.ndarray, q_int_weight: np.ndarray, scale: np.ndarray,
                zero_point: np.ndarray, bias: np.ndarray):
    """Host-side prep: dequantize w to fp16 in panel layout, pack x
    shards mi-major fp16."""
    scale_f = np.float32(np.asarray(scale).reshape(-1)[0])
    zp_f = np.float32(np.asarray(zero_point).reshape(-1)[0])
    w16 = ((np.asarray(q_int_weight).astype(np.float32) - zp_f)
           * scale_f).astype(np.float16)
    # [pa, p, ko, n]: element = w16 at (k=ko*128+p, col=pa*512+n)
    wpk = np.ascontiguousarray(
        w16.reshape(KO, P, NPAN, NTILE).transpose(2, 1, 0, 3))
    bf = np.ascontiguousarray(bias.astype(np.float32))
    xf = np.asarray(x, dtype=np.float32).reshape(B * S, DIN)

    in_maps = []
    for c in range(N_CORES):
        xs = xf[c * M_SH:(c + 1) * M_SH].astype(np.float16)
        # [m=(mi mc), k=(ko p)] -> [p, mi, ko, mc]
        xt = np.ascontiguousarray(
            xs.T.reshape(KO, P, MT, P).transpose(1, 2, 0, 3))
        in_maps.append({"xts": xt, "wpk": wpk, "biass": bf})
    return in_maps


def kernel(x: np.ndarray, q_int_weight: np.ndarray, scale: np.ndarray,
           zero_point: np.ndarray, bias: np.ndarray) -> np.ndarray:
    _ensure_paths()
    from concourse.bass_utils import run_bass_kernel_spmd

    nc = _build()
    in_maps = prep_inputs(x, q_int_weight, scale, zero_point, bias)
    res = run_bass_kernel_spmd(nc, in_maps, core_ids=list(range(N_CORES)))

    y = np.empty((B * S, DOUT), np.float32)
    for c in range(N_CORES):
        y[c * M_SH:(c + 1) * M_SH] = res.results[c]["ys"].astype(np.float32)
    return y.reshape(B, S, DOUT)
